# revision 1
# baseline (speedup 1.0000x reference)
"""Causal multi-head attention on 8 Trainium2 NeuronCores.

Full module: x:(2,2048,1024) f32, 16 heads, head_dim 64, causal softmax
(scaled by 1/sqrt(1024)), out = attn(x) @ Wo + bo.

Sharding: core c handles batch b = c // 4 and head group g = c % 4
(4 heads, i.e. 256 columns of Wq/Wk/Wv and 256 rows of Wo). Every core
runs the same program (SPMD); the host sums the 4 per-group partial
output projections per batch and adds the bias.

Per-core kernel layout strategy (all matmuls contract over the SBUF
partition dim; "T" tensors are stored feature-major so no transposes are
needed mid-attention):
  xT   [f=1024, t=2048]  bf16, built via PE transposes + cast on evict
  QT/KT[d=256,  t=2048]  = Wq/Wk as lhsT, xT as rhs  (2 tiles, head pairs)
  V    [t=2048, d=256]   = xT as lhsT, Wv as rhs; stored [128,16,4,65]
                          with a ones column per head (fused softmax sums)
  S^T  [k=128, q=512]    = KT-slice as lhsT, QT-slice as rhs, two heads
                          run on disjoint partition halves (row-tiled)
  P^T  = exp(S^T/32) via ScalarE, causal mask applied on diagonal blocks
  ctxT [d=64|sums, q]    = [V|1] as lhsT, P^T as rhs, accumulated in PSUM
  norm: recip(sums row) -> PE outer-product broadcast -> DVE multiply
  out  [t, 1024]         = ctxT as lhsT, Wo as rhs  (partial; host-summed)
"""

import os

import numpy as np

N = 2048        # tokens per batch
D = 1024        # model dim
HG = 4          # heads per core
HD = 64         # head dim
DG = HG * HD    # 256, feature columns per core
SCALE = 1.0 / 32.0  # 1/sqrt(D); note module scales by sqrt(d_out), not head_dim
NCORES = 8

# tuning knobs (env-overridable for experiments)
OUTER_F32R = os.environ.get("OUTER_F32R", "1") != "0"  # f32r outer products
PT_BUFS = int(os.environ.get("PT_BUFS", "8"))
HOST_XT = os.environ.get("HOST_XT", "0") != "0"      # host passes x already transposed

_CACHE = {}


def _build_nc(repeat=1):
    from contextlib import ExitStack

    import concourse.mybir as mybir
    import concourse.tile as tile
    from concourse import bacc
    from concourse.masks import make_identity

    FP32 = mybir.dt.float32
    F32R = mybir.dt.float32r
    BF16 = mybir.dt.bfloat16
    EXP = mybir.ActivationFunctionType.Exp
    COPY = mybir.ActivationFunctionType.Copy

    NT = N // 128   # 16 token chunks
    NF = D // 128   # 8 feature chunks
    NQ = N // 512   # 4 query blocks

    nc = bacc.Bacc("TRN2", target_bir_lowering=False, debug=False)

    if HOST_XT:
        x_d = nc.dram_tensor("x", [D, N], BF16, kind="ExternalInput").ap()
    else:
        x_d = nc.dram_tensor("x", [N, D], FP32, kind="ExternalInput").ap()
    wq_d = nc.dram_tensor("wq", [D, DG], BF16, kind="ExternalInput").ap()
    wk_d = nc.dram_tensor("wk", [D, DG], BF16, kind="ExternalInput").ap()
    wv_d = nc.dram_tensor("wv", [D, DG], BF16, kind="ExternalInput").ap()
    wo_d = nc.dram_tensor("wo", [DG, D], BF16, kind="ExternalInput").ap()
    out_d = nc.dram_tensor("out", [N, D], FP32, kind="ExternalOutput").ap()

    with tile.TileContext(nc) as tc, ExitStack() as ctx:
        persist = ctx.enter_context(tc.tile_pool(name="persist", bufs=1))
        xpool = ctx.enter_context(tc.tile_pool(name="xpool", bufs=10))
        ptpool = ctx.enter_context(tc.tile_pool(name="ptpool", bufs=PT_BUFS))
        stpool = ctx.enter_context(tc.tile_pool(name="stpool", bufs=4))
        smpool = ctx.enter_context(tc.tile_pool(name="smpool", bufs=4))
        opool = ctx.enter_context(tc.tile_pool(name="opool", bufs=3))
        # PSUM budget (8 banks): proj/transpose/outproj tag "ps"
        # [128,512]x2 = 2 banks; attention S tag "ps_s" [128,1024]x2 = 4
        # banks (independent rotation domains so the phases pipeline);
        # pv pool 2 banks, slots reused for the broadcast outer-products.
        mmpsum = ctx.enter_context(tc.tile_pool(name="mmpsum", bufs=2, space="PSUM"))
        spsum = mmpsum
        pvpsum = ctx.enter_context(tc.tile_pool(name="pvpsum", bufs=1, space="PSUM"))

        # ---- persistent tensors ----
        xT = persist.tile([128, NF, N], BF16, name="xT")          # 32 KB/p
        qt = persist.tile([128, 2, N], BF16, name="qt")           # 8 KB/p
        kt = persist.tile([128, 2, N], BF16, name="kt")           # 8 KB/p
        vt = persist.tile([128, NT, HG, HD + 1], BF16, name="vt")  # ~8 KB/p
        ctxT = persist.tile([128, 2, N], BF16, name="ctxT")       # 8 KB/p
        wq_bf = persist.tile([128, NF, DG], BF16, name="wq_bf")   # 4 KB/p
        wk_bf = persist.tile([128, NF, DG], BF16, name="wk_bf")
        wv_bf = persist.tile([128, NF, DG], BF16, name="wv_bf")
        wo_bf = persist.tile([128, 2, D], BF16, name="wo_bf")     # 4 KB/p
        ones128 = persist.tile([128, HD], F32R if OUTER_F32R else FP32,
                               name="ones128")
        if not HOST_XT:
            ident = persist.tile([128, 128], FP32, name="ident")
            make_identity(nc, ident[:, :])

        if OUTER_F32R:
            # walrus requires f32r operands produced by a rounding op
            ones_f32 = persist.tile([128, HD], FP32, name="ones_f32")
            nc.gpsimd.memset(ones_f32[:, :], 1.0)
            nc.vector.tensor_copy(ones128[:, :], ones_f32[:, :])
        else:
            nc.gpsimd.memset(ones128[:, :], 1.0)
        nc.gpsimd.memset(vt[:, :, :, HD], 1.0)  # softmax-sum ones columns

        def emit_weights():
            # ---- weights arrive bf16 from the host: straight DMAs ----
            for w_dram, w_bf in ((wq_d, wq_bf), (wk_d, wk_bf), (wv_d, wv_bf)):
                nc.sync.dma_start(out=w_bf[:, :, :],
                                  in_=w_dram.rearrange("(c p) d -> p c d", p=128))
            nc.sync.dma_start(out=wo_bf[:, :, :],
                              in_=wo_d.rearrange("(c p) d -> p c d", p=128))

        def emit_xt_block(ib):
            """Build the bf16 xT slab for one 512-token range."""
            if HOST_XT:
                # x arrives feature-major in bf16: straight DMA into xT
                nc.sync.dma_start(
                    out=xT[:, :, 512 * ib:512 * (ib + 1)],
                    in_=x_d.rearrange("(c p) t -> p c t", p=128)[
                        :, :, 512 * ib:512 * (ib + 1)],
                )
                return
            for u in range(4):
                ti = 4 * ib + u
                t0 = ti * 128
                xn_t = xpool.tile([128, D], FP32, name="xn")
                nc.sync.dma_start(out=xn_t[:, :], in_=x_d[t0:t0 + 128, :])
                for jh in range(2):        # f chunks [4jh .. 4jh+3]
                    ps_tr = mmpsum.tile([128, 512], FP32, name="ps",
                                        tag="ps")
                    for v in range(4):
                        j = 4 * jh + v
                        nc.tensor.transpose(
                            ps_tr[:, 128 * v:128 * (v + 1)],
                            xn_t[:, 128 * j:128 * (j + 1)],
                            ident[:, :],
                        )
                    nc.vector.tensor_copy(
                        xT[:, 4 * jh:4 * jh + 4, 128 * ti:128 * (ti + 1)],
                        ps_tr[:, :].rearrange("p (j t) -> p j t", j=4))

        def emit_proj_mms(ib):
            """Project one 512-token range of xT into QT/KT/V."""
            tb = ib
            for w_bf, dst in ((wq_bf, qt), (wk_bf, kt)):
                for dh in range(2):
                    ps = mmpsum.tile([128, 512], FP32, name="ps", tag="ps")
                    for fc in range(NF):
                        nc.tensor.matmul(
                            ps[:, :],
                            lhsT=w_bf[:, fc, 128 * dh:128 * (dh + 1)],
                            rhs=xT[:, fc, 512 * tb:512 * (tb + 1)],
                            start=(fc == 0), stop=(fc == NF - 1),
                        )
                    nc.vector.tensor_copy(
                        dst[:, dh, 512 * tb:512 * (tb + 1)], ps[:, :])
            for tcc in range(4 * ib, 4 * ib + 4):
                ps = mmpsum.tile([128, 512], FP32, name="ps", tag="ps")
                for fc in range(NF):
                    nc.tensor.matmul(
                        ps[:, 0:DG],
                        lhsT=xT[:, fc, 128 * tcc:128 * (tcc + 1)],
                        rhs=wv_bf[:, fc, :],
                        start=(fc == 0), stop=(fc == NF - 1),
                    )
                nc.vector.tensor_copy(
                    vt[:, tcc, :, 0:HD],
                    ps[:, 0:DG].rearrange("p (h e) -> p h e", h=HG))

        def emit_attention(qb):
            """Attention for one 512-wide query block, both head pairs,
            then the output projection for the same token range."""
            nkc = 4 * (qb + 1)             # causal: k chunks 0..4qb+3
            qsl = slice(512 * qb, 512 * (qb + 1))
            for p in range(2):             # head pair (heads 2p, 2p+1)
                # S + exp + PV stream (PV lags one chunk behind exp)
                pv_a = pvpsum.tile([HD + 1, 512], FP32, name="pv_a",
                                   tag="pv_a")
                pv_b = pvpsum.tile([HD + 1, 512], FP32, name="pv_b",
                                   tag="pv_b")
                for kc in range(nkc):
                    ksl = slice(128 * kc, 128 * (kc + 1))
                    # columns q_local < 128*m are entirely above the
                    # causal diagonal for this k chunk: skip them.
                    m = max(0, kc - 4 * qb)
                    q0 = 128 * m
                    ps_s = spsum.tile([128, 1024], FP32, name="ps_s",
                                      tag="ps_s", bufs=2)
                    # head A on partitions 0-63, head B on 64-127
                    for i in range(2):
                        lo = 64 * i
                        nc.tensor.matmul(
                            ps_s[:, 512 * i:512 * (i + 1)],
                            lhsT=kt[lo:lo + 64, p, ksl],
                            rhs=qt[lo:lo + 64, p, qsl],
                            start=True, stop=True,
                        )
                    pt = ptpool.tile([128, 1024], BF16, name="pt")
                    # full-width exp even on narrowed diagonal chunks: the
                    # skipped columns hold stale PSUM (finite, O(1)) and
                    # are never read downstream
                    nc.scalar.activation(pt[:, :], ps_s[:, :], EXP,
                                         scale=SCALE)
                    if kc >= 4 * qb:       # diagonal: zero q < k in
                        # place on the columns PV will actually read
                        for i in range(2):
                            sl = slice(512 * i + q0, 512 * (i + 1))
                            nc.gpsimd.affine_select(
                                out=pt[:, sl], in_=pt[:, sl],
                                compare_op=mybir.AluOpType.is_ge,
                                fill=0.0,
                                base=0,
                                pattern=[[1, 512 - q0]],
                                channel_multiplier=-1,
                            )
                    st = (kc == 0)
                    sp = (kc == nkc - 1)
                    nc.tensor.matmul(
                        pv_a[:, q0:512], lhsT=vt[:, kc, 2 * p, :],
                        rhs=pt[:, q0:512], start=st, stop=sp,
                    )
                    nc.tensor.matmul(
                        pv_b[:, q0:512], lhsT=vt[:, kc, 2 * p + 1, :],
                        rhs=pt[:, 512 + q0:1024], start=st, stop=sp,
                    )
                # epilogue: stage PSUM out (frees pv slots for the bc
                # outer-products), then normalize by the fused sums row
                st_a = stpool.tile([HD + 1, 512], FP32, name="st_a", tag="st")
                st_b = stpool.tile([HD + 1, 512], FP32, name="st_b", tag="st")
                nc.vector.tensor_copy(st_a[:, :], pv_a[:, :])
                nc.vector.tensor_copy(st_b[:, :], pv_b[:, :])
                rec = smpool.tile([HD + 1, 1024],
                                  F32R if OUTER_F32R else FP32, name="rec")
                with nc.allow_low_precision(reason="f32r softmax recip"):
                    nc.vector.reciprocal(rec[HD:HD + 1, 0:512],
                                         st_a[HD:HD + 1, :])
                    nc.vector.reciprocal(rec[HD:HD + 1, 512:1024],
                                         st_b[HD:HD + 1, :])
                bc_a = pvpsum.tile([HD, 512], FP32, name="bc_a", tag="pv_a")
                bc_b = pvpsum.tile([HD, 512], FP32, name="bc_b", tag="pv_b")
                ones_ap = ones128[HD:HD + 1, :]
                rec_a = rec[HD:HD + 1, 0:512]
                rec_b = rec[HD:HD + 1, 512:1024]
                nc.tensor.matmul(bc_a[:, :], lhsT=ones_ap, rhs=rec_a,
                                 start=True, stop=True)
                nc.tensor.matmul(bc_b[:, :], lhsT=ones_ap, rhs=rec_b,
                                 start=True, stop=True)
                # head A lands on ctxT partitions 0-63 directly
                nc.vector.tensor_mul(ctxT[0:HD, p, qsl], st_a[0:HD, :],
                                     bc_a[:, :])
                # head B: multiply at partitions 0-63, DMA to 64-127
                cb = stpool.tile([HD, 512], BF16, name="cb", tag="cb")
                nc.vector.tensor_mul(cb[:, :], st_b[0:HD, :], bc_b[:, :])
                nc.sync.dma_start(out=ctxT[HD:128, p, qsl], in_=cb[:, :])
            # output projection for this token range (partial over heads)
            for tb in range(4 * qb, 4 * qb + 4):
                tsl = slice(128 * tb, 128 * (tb + 1))
                for nh in range(2):
                    ps_o = mmpsum.tile([128, 512], FP32, name="ps", tag="ps")
                    for hc in range(2):
                        nc.tensor.matmul(
                            ps_o[:, :],
                            lhsT=ctxT[:, hc, tsl],
                            rhs=wo_bf[:, hc, 512 * nh:512 * (nh + 1)],
                            start=(hc == 0), stop=(hc == 1),
                        )
                    o_sb = opool.tile([128, 512], FP32, name="o_sb")
                    # last query block: exps are done by then, ScalarE idle
                    if nh == 0 and qb != NQ - 1:
                        nc.vector.tensor_copy(o_sb[:, :], ps_o[:, :])
                    else:
                        nc.scalar.activation(o_sb[:, :], ps_o[:, :], COPY)
                    nc.sync.dma_start(
                        out=out_d[tsl, 512 * nh:512 * (nh + 1)],
                        in_=o_sb[:, :])

        def emit_body():
            emit_xt_block(0)
            emit_weights()
            emit_proj_mms(0)
            emit_xt_block(1)
            emit_proj_mms(1)
            emit_attention(0)
            emit_xt_block(2)
            emit_proj_mms(2)
            emit_attention(1)
            emit_xt_block(3)
            emit_proj_mms(3)
            emit_attention(2)
            emit_attention(3)

        for _rep in range(repeat):
            emit_body()

    nc.compile()
    return nc


def _get_nc(repeat=1):
    key = ("nc", repeat)
    if key not in _CACHE:
        _CACHE[key] = _build_nc(repeat)
    return _CACHE[key]


def _make_in_maps(x, Wq, Wk, Wv, Wo):
    in_maps = []
    for c in range(NCORES):
        b, g = divmod(c, 4)
        cs = slice(DG * g, DG * (g + 1))
        if HOST_XT:
            import ml_dtypes
            xb = np.ascontiguousarray(x[b].T).astype(ml_dtypes.bfloat16)
        else:
            xb = np.ascontiguousarray(x[b], dtype=np.float32)
        import ml_dtypes
        bf = ml_dtypes.bfloat16
        in_maps.append({
            "x": xb,
            "wq": np.ascontiguousarray(Wq[:, cs]).astype(bf),
            "wk": np.ascontiguousarray(Wk[:, cs]).astype(bf),
            "wv": np.ascontiguousarray(Wv[:, cs]).astype(bf),
            "wo": np.ascontiguousarray(Wo[cs, :]).astype(bf),
        })
    return in_maps


def _gather(results, bo):
    out = np.empty((2, N, D), dtype=np.float32)
    for b in range(2):
        acc = results[4 * b]["out"].astype(np.float32)
        for g in range(1, 4):
            acc = acc + results[4 * b + g]["out"]
        out[b] = acc + bo[None, :].astype(np.float32)
    return out


def run_spmd(x, Wq, Wk, Wv, Wo, bo, **spmd_kwargs):
    """Run the 8-core kernel; returns (full_output, BassKernelResults)."""
    from concourse.bass_utils import run_bass_kernel_spmd

    nc = _get_nc()
    in_maps = _make_in_maps(
        np.asarray(x), np.asarray(Wq), np.asarray(Wk), np.asarray(Wv),
        np.asarray(Wo))
    res = run_bass_kernel_spmd(nc, in_maps, core_ids=list(range(NCORES)),
                               **spmd_kwargs)
    return _gather(res.results, np.asarray(bo)), res


def kernel(x, Wq, Wk, Wv, Wo, bo):
    out, _ = run_spmd(x, Wq, Wk, Wv, Wo, bo)
    return out



# revision 2
# speedup vs baseline: 1.5660x; 1.5660x over previous
"""Causal multi-head attention on 8 Trainium2 NeuronCores (v2, fp8 DoubleRow).

Full module: x:(2,2048,1024) f32, 16 heads, head_dim 64, causal softmax
(scaled by 1/sqrt(1024)), out = attn(x) @ Wo + bo.

Sharding: core c handles batch b = c // 4 and head group g = c % 4
(4 heads, i.e. 256 columns of Wq/Wk/Wv and 256 rows of Wo). Every core
runs the same program (SPMD); the host sums the 4 per-group partial
output projections per batch (f16 partials) and adds the bias.

v2 design vs baseline:
  * host supplies x pre-transposed: xt bf16 [128,8,N] (V proj) and
    x8 fp8e4m3 [128,4,2,N] (Q/K proj, DoubleRow k-tile-pair layout) --
    no PE transposes on device.
  * Q/K projections run fp8 DoubleRow (2 f-chunks per pass, 0.5 cyc/row):
    host scales Wq/Wk by 32 (fp8-friendly range) and permutes their
    columns so the projection psum comes out partition-ordered as
    head*32+d_local with d-halves split across the two dh matmuls.
  * S = K^T Q runs fp8 DoubleRow per head: qt8/kt8 [32h:32h+32, 2, N]
    hold head h's d-halves as the 2 k-tiles; explicit tile_position
    (32h, 0) row-tiles the PE array. exp scale absorbs the 32*32 weight
    scaling (exp(S_psum / 32768)).
  * exp activations trimmed to the causal-valid column range; the
    causal mask (gpsimd affine_select, [128,128] diagonal block only)
    is issued right after exp and consumed by PV one chunk later, so
    Pool latency stays off the critical path. PV of diagonal chunks is
    split into a clean part (no mask dep) and the masked 128-col block.
  * proj / out-proj / epilogue-normalize steps live in a global work
    queue; one step is popped between S(kc) and PV(kc-1) of every
    attention chunk to fill PE slack while ScalarE runs exp.
  * V path / PV / out-proj stay bf16 (fp8 there breaks the 2e-2 gate).
  * out is written as f16 partials (halves output DMA); host upcasts.
"""

import os

import numpy as np

N = 2048        # tokens per batch
D = 1024        # model dim
HG = 4          # heads per core
HD = 64         # head dim
DG = HG * HD    # 256, feature columns per core
NCORES = 8
NT = N // 128   # 16 token chunks
NF = D // 128   # 8 feature chunks
NQ = N // 512   # 4 query blocks
# host scales Wq,Wk by 32 -> S_psum = 1024 * S; module scale 1/sqrt(1024)
EXP_SCALE = 1.0 / (32.0 * 1024.0)

PT_BUFS = int(os.environ.get("PT_BUFS", "8"))

_CACHE = {}


def _build_nc(repeat=1):
    from contextlib import ExitStack

    import concourse.mybir as mybir
    import concourse.tile as tile
    from concourse import bacc

    FP32 = mybir.dt.float32
    F32R = mybir.dt.float32r
    FP16 = mybir.dt.float16
    BF16 = mybir.dt.bfloat16
    F8 = mybir.dt.float8e4
    EXP = mybir.ActivationFunctionType.Exp
    COPY = mybir.ActivationFunctionType.Copy
    DR = mybir.MatmulPerfMode.DoubleRow

    nc = bacc.Bacc("TRN2", target_bir_lowering=False, debug=False)

    x8_d = nc.dram_tensor("x8", [128, NF // 2, 2, N], F8, kind="ExternalInput").ap()
    xt_d = nc.dram_tensor("xt", [128, NF, N], BF16, kind="ExternalInput").ap()
    wq_d = nc.dram_tensor("wq8", [128, NF // 2, 2, DG], F8, kind="ExternalInput").ap()
    wk_d = nc.dram_tensor("wk8", [128, NF // 2, 2, DG], F8, kind="ExternalInput").ap()
    wv_d = nc.dram_tensor("wv", [128, NF, DG], BF16, kind="ExternalInput").ap()
    wo_d = nc.dram_tensor("wo", [128, 2, D], BF16, kind="ExternalInput").ap()
    out_d = nc.dram_tensor("out", [N, D], FP16, kind="ExternalOutput").ap()

    with tile.TileContext(nc) as tc, ExitStack() as ctx:
        persist = ctx.enter_context(tc.tile_pool(name="persist", bufs=1))
        ptpool = ctx.enter_context(tc.tile_pool(name="ptpool", bufs=PT_BUFS))
        stpool = ctx.enter_context(tc.tile_pool(name="stpool", bufs=4))
        smpool = ctx.enter_context(tc.tile_pool(name="smpool", bufs=4))
        opool = ctx.enter_context(tc.tile_pool(name="opool", bufs=3))
        # PSUM budget (8 banks): proj/outproj/bc "ps" [128,512]x2 = 2 banks;
        # attention S "ps_s" [128,1024]x2 = 4 banks; pv accumulators 2 banks.
        mmpsum = ctx.enter_context(tc.tile_pool(name="mmpsum", bufs=2, space="PSUM"))
        spsum = mmpsum
        pvpsum = ctx.enter_context(tc.tile_pool(name="pvpsum", bufs=1, space="PSUM"))

        # ---- persistent tensors ----
        x8 = persist.tile([128, NF // 2, 2, N], F8, name="x8")    # 16 KB/p
        xT = persist.tile([128, NF, N], BF16, name="xT")          # 32 KB/p
        qt8 = persist.tile([128, 2, N], F8, name="qt8")           # 4 KB/p
        kt8 = persist.tile([128, 2, N], F8, name="kt8")           # 4 KB/p
        vt = persist.tile([128, NT, HG, HD + 1], BF16, name="vt")  # ~8 KB/p
        ctxT = persist.tile([128, 2, N], BF16, name="ctxT")       # 8 KB/p
        wq8 = persist.tile([128, NF // 2, 2, DG], F8, name="wq8")  # 2 KB/p
        wk8 = persist.tile([128, NF // 2, 2, DG], F8, name="wk8")
        wv_bf = persist.tile([128, NF, DG], BF16, name="wv_bf")   # 4 KB/p
        wo_bf = persist.tile([128, 2, D], BF16, name="wo_bf")     # 4 KB/p
        ones128 = persist.tile([128, HD], F32R, name="ones128")

        # walrus requires f32r operands produced by a rounding op
        ones_f32 = persist.tile([128, HD], FP32, name="ones_f32")
        nc.gpsimd.memset(ones_f32[:, :], 1.0)
        nc.vector.tensor_copy(ones128[:, :], ones_f32[:, :])
        nc.gpsimd.memset(vt[:, :, :, HD], 1.0)  # softmax-sum ones columns

        def emit_weights_vo():
            nc.sync.dma_start(out=wv_bf[:, :, :], in_=wv_d)
            nc.sync.dma_start(out=wo_bf[:, :, :], in_=wo_d)

        def emit_x8_dma(ib, half=None):
            if half is None:
                tsl = slice(512 * ib, 512 * (ib + 1))
            else:
                tsl = slice(512 * ib + 256 * half, 512 * ib + 256 * (half + 1))
            nc.sync.dma_start(out=x8[:, :, :, tsl], in_=x8_d[:, :, :, tsl])

        def emit_xt_dma(ib):
            tsl = slice(512 * ib, 512 * (ib + 1))
            nc.sync.dma_start(out=xT[:, :, tsl], in_=xt_d[:, :, tsl])

        def make_qk_steps(ib):
            """4 steps: (Q|K) x (dh 0|1). th outer so the first half of
            the x8 slab suffices to start."""
            tsl = slice(512 * ib, 512 * (ib + 1))
            steps = []

            def qk_step(w8, dst, dh):
                def go():
                    ps = mmpsum.tile([128, 512], FP32, name="ps", tag="ps")
                    for th in range(2):
                        for j in range(NF // 2):
                            nc.tensor.matmul(
                                ps[:, 256 * th:256 * (th + 1)],
                                lhsT=w8[:, j, :, 128 * dh:128 * (dh + 1)],
                                rhs=x8[:, j, :,
                                       512 * ib + 256 * th:
                                       512 * ib + 256 * (th + 1)],
                                start=(j == 0), stop=(j == NF // 2 - 1),
                                perf_mode=DR,
                            )
                    nc.vector.tensor_copy(dst[:, dh, tsl], ps[:, :])
                return go

            for w8, dst in ((wq8, qt8), (wk8, kt8)):
                for dh in range(2):
                    steps.append(qk_step(w8, dst, dh))
            return steps

        def make_v_steps(ib, tccs=None):
            steps = []

            def v_step(tcc):
                def go():
                    ps = mmpsum.tile([128, 512], FP32, name="ps", tag="ps")
                    for fc in range(NF):
                        nc.tensor.matmul(
                            ps[:, 0:DG],
                            lhsT=xT[:, fc, 128 * tcc:128 * (tcc + 1)],
                            rhs=wv_bf[:, fc, :],
                            start=(fc == 0), stop=(fc == NF - 1),
                        )
                    # (gpsimd cannot read PSUM -- must stay on DVE)
                    nc.vector.tensor_copy(
                        vt[:, tcc, :, 0:HD],
                        ps[:, 0:DG].rearrange("p (h e) -> p h e", h=HG))
                return go

            for tcc in (tccs if tccs is not None
                        else range(4 * ib, 4 * ib + 4)):
                steps.append(v_step(tcc))
            return steps

        def make_outproj_steps(qb):
            """8 steps: (tb, nh); partial over heads, host sums groups."""
            steps = []

            def o_step(tb, nh):
                def go():
                    tsl = slice(128 * tb, 128 * (tb + 1))
                    ps_o = mmpsum.tile([128, 512], FP32, name="ps", tag="ps")
                    for hc in range(2):
                        nc.tensor.matmul(
                            ps_o[:, :],
                            lhsT=ctxT[:, hc, tsl],
                            rhs=wo_bf[:, hc, 512 * nh:512 * (nh + 1)],
                            start=(hc == 0), stop=(hc == 1),
                        )
                    o_sb = opool.tile([128, 512], FP16, name="o_sb")
                    # tail block: exps are done, alternate ACT/DVE eviction
                    # and spread the out DMAs over a second queue
                    if qb == NQ - 1 and nh == 0:
                        nc.scalar.activation(o_sb[:, :], ps_o[:, :], COPY)
                    else:
                        nc.vector.tensor_copy(o_sb[:, :], ps_o[:, :])
                    eng = nc.scalar if qb == NQ - 1 and nh == 1 else nc.sync
                    eng.dma_start(
                        out=out_d[tsl, 512 * nh:512 * (nh + 1)],
                        in_=o_sb[:, :])
                return go

            for tb in range(4 * qb, 4 * qb + 4):
                for nh in range(2):
                    steps.append(o_step(tb, nh))
            return steps

        def emit_attention(qb, queue):
            """Attention for one 512-wide query block, both head pairs.
            Pops one queue step per chunk (between S(kc) and PV(kc-1))
            to fill PE slack while ScalarE runs the exps."""
            nkc = 4 * (qb + 1)             # causal: k chunks 0..4qb+3
            qsl = slice(512 * qb, 512 * (qb + 1))

            def emit_S_exp_mask(p, kc):
                m = max(0, kc - 4 * qb)
                q0 = 128 * m
                ps_s = spsum.tile([128, 1024], FP32, name="ps_s",
                                  tag="ps_s", bufs=2)
                ksl = slice(128 * kc, 128 * (kc + 1))
                for hh in range(2):
                    h = 2 * p + hh
                    pb = 32 * h
                    for th in range(2):
                        # note: fully-masked th ranges are still computed;
                        # skipping them leaves psum bytes unwritten and the
                        # trailing exp read would race with the previous
                        # tile generation's writers
                        nc.tensor.matmul(
                            ps_s[:, 512 * hh + 256 * th:
                                 512 * hh + 256 * (th + 1)],
                            lhsT=kt8[pb:pb + 32, :, ksl],
                            rhs=qt8[pb:pb + 32, :,
                                    512 * qb + 256 * th:
                                    512 * qb + 256 * (th + 1)],
                            start=True, stop=True,
                            perf_mode=DR,
                            tile_position=(pb, 0),
                        )
                pt = ptpool.tile([128, 1024], BF16, name="pt")
                # single contiguous exp over [q0, 1024): covers head A's
                # valid cols [q0,512) and head B's [512+q0,1024); cols
                # [512,512+q0) hold exp(stale psum) and are never read
                nc.scalar.activation(pt[:, q0:1024], ps_s[:, q0:1024],
                                     EXP, scale=EXP_SCALE)
                diag = kc >= 4 * qb
                if diag:
                    # triangular mask only touches the 128-col diagonal
                    # block [q0, q0+128); later cols are fully valid
                    for i in range(2):
                        sl = slice(512 * i + q0, 512 * i + q0 + 128)
                        nc.gpsimd.affine_select(
                            out=pt[:, sl], in_=pt[:, sl],
                            compare_op=mybir.AluOpType.is_ge,
                            fill=0.0,
                            base=0,
                            pattern=[[1, 128]],
                            channel_multiplier=-1,
                        )
                return pt, q0, diag

            def emit_PV(p, kc, pt, q0, diag, pv_a, pv_b):
                # exactly one start (first instr of kc==0) and one stop
                # (last instr of kc==nkc-1) per accumulator: psum "start"
                # marks the whole 2KB zero region pending-zero, so later
                # first-touches of other columns still get zeroed.
                st = (kc == 0)
                last = (kc == nkc - 1)
                for hh, pv in ((0, pv_a), (1, pv_b)):
                    base = 512 * hh
                    if diag:
                        if q0 + 128 < 512:   # clean part, no mask dep
                            nc.tensor.matmul(
                                pv[:, q0 + 128:512],
                                lhsT=vt[:, kc, 2 * p + hh, :],
                                rhs=pt[:, base + q0 + 128:base + 512],
                                start=st, stop=False,
                            )
                            st = False
                        # masked 128-col diagonal block
                        nc.tensor.matmul(
                            pv[:, q0:q0 + 128],
                            lhsT=vt[:, kc, 2 * p + hh, :],
                            rhs=pt[:, base + q0:base + q0 + 128],
                            start=st, stop=last,
                        )
                    else:
                        nc.tensor.matmul(
                            pv[:, 0:512],
                            lhsT=vt[:, kc, 2 * p + hh, :],
                            rhs=pt[:, base:base + 512],
                            start=st, stop=False,
                        )
                    st = (kc == 0)   # reset for the hh=1 accumulator

            def make_epilogue2(p, st_a, st_b):
                def go():
                    rec = smpool.tile([HD + 1, 1024], F32R, name="rec")
                    with nc.allow_low_precision(reason="f32r softmax recip"):
                        nc.vector.reciprocal(rec[HD:HD + 1, 0:512],
                                             st_a[HD:HD + 1, :])
                        nc.vector.reciprocal(rec[HD:HD + 1, 512:1024],
                                             st_b[HD:HD + 1, :])
                    bc_a = mmpsum.tile([HD, 512], FP32, name="bc_a", tag="ps")
                    bc_b = mmpsum.tile([HD, 512], FP32, name="bc_b", tag="ps")
                    ones_ap = ones128[HD:HD + 1, :]
                    nc.tensor.matmul(bc_a[:, :], lhsT=ones_ap,
                                     rhs=rec[HD:HD + 1, 0:512],
                                     start=True, stop=True)
                    nc.tensor.matmul(bc_b[:, :], lhsT=ones_ap,
                                     rhs=rec[HD:HD + 1, 512:1024],
                                     start=True, stop=True)
                    # head 2p lands on ctxT partitions 0-63 directly
                    nc.vector.tensor_mul(ctxT[0:HD, p, qsl], st_a[0:HD, :],
                                         bc_a[:, :])
                    # head 2p+1: multiply at partitions 0-63, DMA to 64-127
                    cb = stpool.tile([HD, 512], BF16, name="cb", tag="cb")
                    nc.vector.tensor_mul(cb[:, :], st_b[0:HD, :], bc_b[:, :])
                    # gpsimd-issued DMA: keeps the latency-critical ctxT
                    # relocation off the busy sync queue
                    nc.gpsimd.dma_start(out=ctxT[HD:128, p, qsl],
                                        in_=cb[:, :])
                return go

            for p in range(2):             # head pair (heads 2p, 2p+1)
                pv_a = pvpsum.tile([HD + 1, 512], FP32, name="pv_a",
                                   tag="pv_a")
                pv_b = pvpsum.tile([HD + 1, 512], FP32, name="pv_b",
                                   tag="pv_b")
                pending = None
                for kc in range(nkc):
                    pt, q0, diag = emit_S_exp_mask(p, kc)
                    if pending is not None:
                        if queue:
                            queue.pop(0)()   # fill exp-wait with queued work
                        emit_PV(p, *pending, pv_a, pv_b)
                    pending = (kc, pt, q0, diag)
                if queue:
                    queue.pop(0)()
                emit_PV(p, *pending, pv_a, pv_b)

                # epilogue stage 1: stage PSUM out (frees pv slots);
                # stage 2 (normalize into ctxT) is deferred to the queue
                st_a = stpool.tile([HD + 1, 512], FP32, name="st_a", tag="st")
                st_b = stpool.tile([HD + 1, 512], FP32, name="st_b", tag="st")
                nc.vector.tensor_copy(st_a[:, :], pv_a[:, :])
                nc.vector.tensor_copy(st_b[:, :], pv_b[:, :])
                queue.insert(0, make_epilogue2(p, st_a, st_b))

        def emit_body():
            queue = []
            # DMA priority: the first QK step needs wq8 + x8(0) only
            nc.sync.dma_start(out=wq8[:, :, :, :], in_=wq_d)
            emit_x8_dma(0, half=0)
            emit_x8_dma(0, half=1)
            nc.sync.dma_start(out=wk8[:, :, :, :], in_=wk_d)
            emit_xt_dma(0)
            emit_weights_vo()
            emit_x8_dma(1)
            emit_xt_dma(1)
            for s in make_qk_steps(0):
                s()
            for s in make_v_steps(0, tccs=[0]):
                s()
            queue += make_v_steps(0, tccs=[1, 2, 3]) + make_qk_steps(1) \
                + make_v_steps(1)
            emit_x8_dma(2)
            emit_xt_dma(2)
            emit_attention(0, queue)
            emit_x8_dma(3)
            emit_xt_dma(3)
            queue += make_qk_steps(2) + make_outproj_steps(0)
            emit_attention(1, queue)
            # v(2)/v(3) deferred into their own attention blocks: PV only
            # touches vt[kc] ~kc chunks in, so the pops land in time
            queue += make_v_steps(2) + make_qk_steps(3) + make_outproj_steps(1)
            emit_attention(2, queue)
            queue += make_v_steps(3) + make_outproj_steps(2)
            emit_attention(3, queue)
            queue += make_outproj_steps(3)
            while queue:
                queue.pop(0)()

        for _rep in range(repeat):
            emit_body()

    nc.compile()
    return nc


def _get_nc(repeat=1):
    key = ("nc", repeat)
    if key not in _CACHE:
        _CACHE[key] = _build_nc(repeat)
    return _CACHE[key]


def _np_f8():
    import concourse.mybir as mybir
    return mybir.dt.np(mybir.dt.float8e4)


def _make_in_maps(x, Wq, Wk, Wv, Wo):
    import ml_dtypes
    bf = ml_dtypes.bfloat16
    f8 = _np_f8()
    x = np.asarray(x, dtype=np.float32)
    in_maps = []

    def dr_w(Wg):
        """[1024, 256] -> fp8 DoubleRow layout [128, 4, 2, 256] with
        columns permuted to (d_half, head, d_local%32)."""
        Wp = Wg.reshape(D, HG, 2, 32).transpose(0, 2, 1, 3).reshape(D, DG)
        return np.ascontiguousarray(
            Wp.reshape(NF // 2, 2, 128, DG).transpose(2, 0, 1, 3)).astype(f8)

    for c in range(NCORES):
        b, g = divmod(c, 4)
        cs = slice(DG * g, DG * (g + 1))
        xT = np.ascontiguousarray(x[b].T)                       # [1024, N]
        xt_bf = np.ascontiguousarray(
            xT.reshape(NF, 128, N).transpose(1, 0, 2)).astype(bf)
        x8 = np.ascontiguousarray(
            xT.reshape(NF // 2, 2, 128, N).transpose(2, 0, 1, 3)).astype(f8)
        wv = np.ascontiguousarray(
            np.asarray(Wv[:, cs], np.float32).reshape(NF, 128, DG)
            .transpose(1, 0, 2)).astype(bf)
        wo = np.ascontiguousarray(
            np.asarray(Wo[cs, :], np.float32).reshape(2, 128, D)
            .transpose(1, 0, 2)).astype(bf)
        in_maps.append({
            "x8": x8,
            "xt": xt_bf,
            "wq8": dr_w(32.0 * np.asarray(Wq[:, cs], np.float32)),
            "wk8": dr_w(32.0 * np.asarray(Wk[:, cs], np.float32)),
            "wv": wv,
            "wo": wo,
        })
    return in_maps


def _gather(results, bo):
    out = np.empty((2, N, D), dtype=np.float32)
    for b in range(2):
        acc = results[4 * b]["out"].astype(np.float32)
        for g in range(1, 4):
            acc = acc + results[4 * b + g]["out"].astype(np.float32)
        out[b] = acc + bo[None, :].astype(np.float32)
    return out


def run_spmd(x, Wq, Wk, Wv, Wo, bo, **spmd_kwargs):
    """Run the 8-core kernel; returns (full_output, BassKernelResults)."""
    from concourse.bass_utils import run_bass_kernel_spmd

    nc = _get_nc()
    in_maps = _make_in_maps(
        np.asarray(x), np.asarray(Wq), np.asarray(Wk), np.asarray(Wv),
        np.asarray(Wo))
    res = run_bass_kernel_spmd(nc, in_maps, core_ids=list(range(NCORES)),
                               **spmd_kwargs)
    return _gather(res.results, np.asarray(bo)), res


def kernel(x, Wq, Wk, Wv, Wo, bo):
    out, _ = run_spmd(x, Wq, Wk, Wv, Wo, bo)
    return out


# revision 3
# speedup vs baseline: 2.2219x; 1.4189x over previous
"""Causal multi-head attention on 8 Trainium2 NeuronCores (v2, fp8 DoubleRow).

Full module: x:(2,2048,1024) f32, 16 heads, head_dim 64, causal softmax
(scaled by 1/sqrt(1024)), out = attn(x) @ Wo + bo.

Sharding: core c handles batch b = c // 4 and head group g = c % 4
(4 heads, i.e. 256 columns of Wq/Wk/Wv and 256 rows of Wo). Every core
runs the same program (SPMD); the host sums the 4 per-group partial
output projections per batch (f16 partials) and adds the bias.

v2 design vs baseline:
  * host supplies x pre-transposed: xt bf16 [128,8,N] (V proj) and
    x8 fp8e4m3 [128,4,2,N] (Q/K proj, DoubleRow k-tile-pair layout) --
    no PE transposes on device.
  * Q/K projections run fp8 DoubleRow (2 f-chunks per pass, 0.5 cyc/row):
    host scales Wq/Wk by 32 (fp8-friendly range) and permutes their
    columns so the projection psum comes out partition-ordered as
    head*32+d_local with d-halves split across the two dh matmuls.
  * S = K^T Q runs fp8 DoubleRow per head: qt8/kt8 [32h:32h+32, 2, N]
    hold head h's d-halves as the 2 k-tiles; explicit tile_position
    (32h, 0) row-tiles the PE array. exp scale absorbs the 32*32 weight
    scaling (exp(S_psum / 32768)).
  * exp activations trimmed to the causal-valid column range; the
    causal mask (gpsimd affine_select, [128,128] diagonal block only)
    is issued right after exp and consumed by PV one chunk later, so
    Pool latency stays off the critical path. PV of diagonal chunks is
    split into a clean part (no mask dep) and the masked 128-col block.
  * proj / out-proj / epilogue-normalize steps live in a global work
    queue; one step is popped between S(kc) and PV(kc-1) of every
    attention chunk to fill PE slack while ScalarE runs exp.
  * V path / PV / out-proj stay bf16 (fp8 there breaks the 2e-2 gate).
  * out is written as f16 partials (halves output DMA); host upcasts.
"""

import os

import numpy as np

N = 2048        # tokens per batch
D = 1024        # model dim
HG = 4          # heads per core
HD = 64         # head dim
DG = HG * HD    # 256, feature columns per core
NCORES = 8
NT = N // 128   # 16 token chunks
NF = D // 128   # 8 feature chunks
NQ = N // 512   # 4 query blocks
# host scales Wq,Wk by 32 -> S_psum = 1024 * S; module scale 1/sqrt(1024)
EXP_SCALE = 1.0 / (32.0 * 1024.0)

PT_BUFS = int(os.environ.get("PT_BUFS", "10"))

_CACHE = {}


def _build_nc(repeat=1):
    from contextlib import ExitStack

    import concourse.mybir as mybir
    import concourse.tile as tile
    from concourse import bacc

    FP32 = mybir.dt.float32
    F32R = mybir.dt.float32r
    FP16 = mybir.dt.float16
    BF16 = mybir.dt.bfloat16
    F8 = mybir.dt.float8e4
    EXP = mybir.ActivationFunctionType.Exp
    COPY = mybir.ActivationFunctionType.Copy
    DR = mybir.MatmulPerfMode.DoubleRow

    nc = bacc.Bacc("TRN2", target_bir_lowering=False, debug=False)

    x8_d = nc.dram_tensor("x8", [128, NF // 2, 2, N], F8, kind="ExternalInput").ap()
    xt_d = nc.dram_tensor("xt", [128, NF, N], BF16, kind="ExternalInput").ap()
    wq_d = nc.dram_tensor("wq8", [128, NF // 2, 2, DG], F8, kind="ExternalInput").ap()
    wk_d = nc.dram_tensor("wk8", [128, NF // 2, 2, DG], F8, kind="ExternalInput").ap()
    wv_d = nc.dram_tensor("wv", [128, NF, DG], BF16, kind="ExternalInput").ap()
    wo_d = nc.dram_tensor("wo", [128, 2, D], BF16, kind="ExternalInput").ap()
    out_d = nc.dram_tensor("out", [N, D], FP16, kind="ExternalOutput").ap()

    with tile.TileContext(nc) as tc, ExitStack() as ctx:
        persist = ctx.enter_context(tc.tile_pool(name="persist", bufs=1))
        ptpool = ctx.enter_context(tc.tile_pool(name="ptpool", bufs=PT_BUFS))
        stpool = ctx.enter_context(tc.tile_pool(name="stpool", bufs=6))
        smpool = ctx.enter_context(tc.tile_pool(name="smpool", bufs=4))
        opool = ctx.enter_context(tc.tile_pool(name="opool", bufs=3))
        # PSUM budget (8 banks): proj/outproj/bc "ps" [128,512]x2 = 2 banks;
        # attention S "ps_s" [128,1024]x2 = 4 banks; pv accumulators 2 banks.
        mmpsum = ctx.enter_context(tc.tile_pool(name="mmpsum", bufs=2, space="PSUM"))
        spsum = mmpsum
        pvpsum = ctx.enter_context(tc.tile_pool(name="pvpsum", bufs=1, space="PSUM"))

        # ---- persistent tensors ----
        x8 = persist.tile([128, NF // 2, 2, N], F8, name="x8")    # 16 KB/p
        xT = persist.tile([128, NF, N], BF16, name="xT")          # 32 KB/p
        qt8 = persist.tile([128, 2, N], F8, name="qt8")           # 4 KB/p
        kt8 = persist.tile([128, 2, N], F8, name="kt8")           # 4 KB/p
        vt = persist.tile([128, NT, HG, HD + 1], BF16, name="vt")  # ~8 KB/p
        ctxT = persist.tile([128, 2, N], BF16, name="ctxT")       # 8 KB/p
        wq8 = persist.tile([128, NF // 2, 2, DG], F8, name="wq8")  # 2 KB/p
        wk8 = persist.tile([128, NF // 2, 2, DG], F8, name="wk8")
        wv_bf = persist.tile([128, NF, DG], BF16, name="wv_bf")   # 4 KB/p
        wo_bf = persist.tile([128, 2, D], BF16, name="wo_bf")     # 4 KB/p
        ones128 = persist.tile([128, HD], F32R, name="ones128")

        # walrus requires f32r operands produced by a rounding op
        ones_f32 = persist.tile([128, HD], FP32, name="ones_f32")
        nc.gpsimd.memset(ones_f32[:, :], 1.0)
        nc.vector.tensor_copy(ones128[:, :], ones_f32[:, :])
        nc.gpsimd.memset(vt[:, :, :, HD], 1.0)  # softmax-sum ones columns

        def emit_weights_vo():
            nc.sync.dma_start(out=wv_bf[:, :, :], in_=wv_d)
            nc.sync.dma_start(out=wo_bf[:, :, :], in_=wo_d)

        def emit_x8_dma(ib, half=None):
            if half is None:
                tsl = slice(512 * ib, 512 * (ib + 1))
            else:
                tsl = slice(512 * ib + 256 * half, 512 * ib + 256 * (half + 1))
            nc.sync.dma_start(out=x8[:, :, :, tsl], in_=x8_d[:, :, :, tsl])

        def emit_xt_dma(ib):
            tsl = slice(512 * ib, 512 * (ib + 1))
            nc.sync.dma_start(out=xT[:, :, tsl], in_=xt_d[:, :, tsl])

        def make_qk_steps(ib, only=None):
            """4 steps: (Q|K) x (dh 0|1). th outer so the first half of
            the x8 slab suffices to start. only='q'|'k' selects half."""
            tsl = slice(512 * ib, 512 * (ib + 1))
            steps = []

            def qk_step(w8, dst, dh):
                def go():
                    ps = mmpsum.tile([128, 512], FP32, name="ps", tag="ps")
                    for th in range(2):
                        for j in range(NF // 2):
                            nc.tensor.matmul(
                                ps[:, 256 * th:256 * (th + 1)],
                                lhsT=w8[:, j, :, 128 * dh:128 * (dh + 1)],
                                rhs=x8[:, j, :,
                                       512 * ib + 256 * th:
                                       512 * ib + 256 * (th + 1)],
                                start=(j == 0), stop=(j == NF // 2 - 1),
                                perf_mode=DR,
                            )
                    nc.vector.tensor_copy(dst[:, dh, tsl], ps[:, :])
                return go

            pairs = {"q": ((wq8, qt8),), "k": ((wk8, kt8),),
                     None: ((wq8, qt8), (wk8, kt8))}[only]
            for w8, dst in pairs:
                for dh in range(2):
                    steps.append(qk_step(w8, dst, dh))
            return steps

        def make_v_steps(ib, tccs=None):
            steps = []

            def v_step(tcc):
                def go():
                    ps = mmpsum.tile([128, 512], FP32, name="ps", tag="ps")
                    for fc in range(NF):
                        nc.tensor.matmul(
                            ps[:, 0:DG],
                            lhsT=xT[:, fc, 128 * tcc:128 * (tcc + 1)],
                            rhs=wv_bf[:, fc, :],
                            start=(fc == 0), stop=(fc == NF - 1),
                        )
                    # (gpsimd cannot read PSUM -- must stay on DVE)
                    nc.vector.tensor_copy(
                        vt[:, tcc, :, 0:HD],
                        ps[:, 0:DG].rearrange("p (h e) -> p h e", h=HG))
                return go

            for tcc in (tccs if tccs is not None
                        else range(4 * ib, 4 * ib + 4)):
                steps.append(v_step(tcc))
            return steps

        def make_outproj_steps(qb):
            """8 steps: (tb, nh); partial over heads, host sums groups."""
            steps = []

            def o_step(tb, nh):
                def go():
                    tsl = slice(128 * tb, 128 * (tb + 1))
                    ps_o = mmpsum.tile([128, 512], FP32, name="ps", tag="ps")
                    for hc in range(2):
                        nc.tensor.matmul(
                            ps_o[:, :],
                            lhsT=ctxT[:, hc, tsl],
                            rhs=wo_bf[:, hc, 512 * nh:512 * (nh + 1)],
                            start=(hc == 0), stop=(hc == 1),
                        )
                    o_sb = opool.tile([128, 512], FP16, name="o_sb")
                    # tail block: exps are done, alternate ACT/DVE eviction
                    # and spread the out DMAs over a second queue
                    if qb == NQ - 1 and nh == 0:
                        nc.scalar.activation(o_sb[:, :], ps_o[:, :], COPY)
                    else:
                        nc.vector.tensor_copy(o_sb[:, :], ps_o[:, :])
                    eng = nc.scalar if qb == NQ - 1 and nh == 1 else nc.sync
                    eng.dma_start(
                        out=out_d[tsl, 512 * nh:512 * (nh + 1)],
                        in_=o_sb[:, :])
                return go

            for tb in range(4 * qb, 4 * qb + 4):
                for nh in range(2):
                    steps.append(o_step(tb, nh))
            return steps

        def emit_attention_stream(queue, prologue):
            """All query blocks' attention chunks as ONE flat stream:
            (qb, p, kc) in causal order, PV lagging S/exp by TWO chunks
            so the next chunk's S (which gates the next exp) always runs
            during the current exp -- ScalarE never waits on queue pops
            or PV. One queue step pops per chunk, after the S. Phase
            boundaries get no pipeline flush; pv accumulators allocate
            lazily at each phase's first PV."""

            def emit_S_exp_mask(qb, p, kc):
                m = max(0, kc - 4 * qb)
                q0 = 128 * m
                ps_s = spsum.tile([128, 1024], FP32, name="ps_s",
                                  tag="ps_s", bufs=2)
                ksl = slice(128 * kc, 128 * (kc + 1))
                for hh in range(2):
                    h = 2 * p + hh
                    pb = 32 * h
                    for th in range(2):
                        if 256 * (th + 1) <= q0:
                            continue   # entire 256-col q-range masked;
                            # the exp below skips these cols too
                        nc.tensor.matmul(
                            ps_s[:, 512 * hh + 256 * th:
                                 512 * hh + 256 * (th + 1)],
                            lhsT=kt8[pb:pb + 32, :, ksl],
                            rhs=qt8[pb:pb + 32, :,
                                    512 * qb + 256 * th:
                                    512 * qb + 256 * (th + 1)],
                            start=True, stop=True,
                            perf_mode=DR,
                            tile_position=(pb, 0),
                        )
                pt = ptpool.tile([128, 1024], BF16, name="pt")
                if q0 < 256:
                    # single contiguous exp over [q0, 1024): covers head
                    # A's valid cols [q0,512) and head B's [512+q0,1024);
                    # cols [512,512+q0) hold exp(stale-but-written psum)
                    # and are never read
                    nc.scalar.activation(pt[:, q0:1024], ps_s[:, q0:1024],
                                         EXP, scale=EXP_SCALE)
                else:
                    # th=0 S instrs were skipped: exp only written ranges
                    for hh in range(2):
                        sl = slice(512 * hh + q0, 512 * (hh + 1))
                        nc.scalar.activation(pt[:, sl], ps_s[:, sl],
                                             EXP, scale=EXP_SCALE)
                diag = kc >= 4 * qb
                if diag:
                    # triangular mask only touches the 128-col diagonal
                    # block [q0, q0+128); later cols are fully valid
                    for i in range(2):
                        sl = slice(512 * i + q0, 512 * i + q0 + 128)
                        nc.gpsimd.affine_select(
                            out=pt[:, sl], in_=pt[:, sl],
                            compare_op=mybir.AluOpType.is_ge,
                            fill=0.0,
                            base=0,
                            pattern=[[1, 128]],
                            channel_multiplier=-1,
                        )
                return pt, q0, diag

            def emit_PV(qb, p, kc, pt, q0, diag, pv_a, pv_b):
                # exactly one start (first instr of kc==0) and one stop
                # (last instr of kc==nkc-1) per accumulator: psum "start"
                # marks the whole 2KB zero region pending-zero, so later
                # first-touches of other columns still get zeroed.
                st = (kc == 0)
                last = (kc == 4 * (qb + 1) - 1)
                for hh, pv in ((0, pv_a), (1, pv_b)):
                    base = 512 * hh
                    if diag:
                        if q0 + 128 < 512:   # clean part, no mask dep
                            nc.tensor.matmul(
                                pv[:, q0 + 128:512],
                                lhsT=vt[:, kc, 2 * p + hh, :],
                                rhs=pt[:, base + q0 + 128:base + 512],
                                start=st, stop=False,
                            )
                            st = False
                        # masked 128-col diagonal block
                        nc.tensor.matmul(
                            pv[:, q0:q0 + 128],
                            lhsT=vt[:, kc, 2 * p + hh, :],
                            rhs=pt[:, base + q0:base + q0 + 128],
                            start=st, stop=last,
                        )
                    else:
                        nc.tensor.matmul(
                            pv[:, 0:512],
                            lhsT=vt[:, kc, 2 * p + hh, :],
                            rhs=pt[:, base:base + 512],
                            start=st, stop=False,
                        )
                    st = (kc == 0)   # reset for the hh=1 accumulator

            def make_epilogue2(qb, p, st_a, st_b):
                qsl = slice(512 * qb, 512 * (qb + 1))

                def go():
                    rec = smpool.tile([HD + 1, 1024], F32R, name="rec")
                    with nc.allow_low_precision(reason="f32r softmax recip"):
                        nc.vector.reciprocal(rec[HD:HD + 1, 0:512],
                                             st_a[HD:HD + 1, :])
                        nc.vector.reciprocal(rec[HD:HD + 1, 512:1024],
                                             st_b[HD:HD + 1, :])
                    bc_a = mmpsum.tile([HD, 512], FP32, name="bc_a", tag="ps")
                    bc_b = mmpsum.tile([HD, 512], FP32, name="bc_b", tag="ps")
                    ones_ap = ones128[HD:HD + 1, :]
                    nc.tensor.matmul(bc_a[:, :], lhsT=ones_ap,
                                     rhs=rec[HD:HD + 1, 0:512],
                                     start=True, stop=True)
                    nc.tensor.matmul(bc_b[:, :], lhsT=ones_ap,
                                     rhs=rec[HD:HD + 1, 512:1024],
                                     start=True, stop=True)
                    # head 2p lands on ctxT partitions 0-63 directly
                    nc.vector.tensor_mul(ctxT[0:HD, p, qsl], st_a[0:HD, :],
                                         bc_a[:, :])
                    # head 2p+1: multiply at partitions 0-63, DMA to 64-127
                    cb = stpool.tile([HD, 512], BF16, name="cb", tag="cb")
                    nc.vector.tensor_mul(cb[:, :], st_b[0:HD, :], bc_b[:, :])
                    # gpsimd-issued DMA: keeps the latency-critical ctxT
                    # relocation off the busy sync queue
                    nc.gpsimd.dma_start(out=ctxT[HD:128, p, qsl],
                                        in_=cb[:, :])
                return go

            pv = {}        # (qb, p) -> (pv_a, pv_b), allocated lazily

            def flush_one():
                qb_, p_, kc_, pt_, q0_, diag_ = pending.pop(0)
                if kc_ == 0:
                    pv[(qb_, p_)] = (
                        pvpsum.tile([HD + 1, 512], FP32, name="pv_a",
                                    tag="pv_a"),
                        pvpsum.tile([HD + 1, 512], FP32, name="pv_b",
                                    tag="pv_b"),
                    )
                pv_a, pv_b = pv[(qb_, p_)]
                emit_PV(qb_, p_, kc_, pt_, q0_, diag_, pv_a, pv_b)
                if kc_ == 4 * (qb_ + 1) - 1:
                    # epilogue stage 1: stage PSUM out (frees pv slots);
                    # stage 2 (normalize into ctxT) goes to the queue head
                    st_a = stpool.tile([HD + 1, 512], FP32, name="st_a",
                                       tag="st")
                    st_b = stpool.tile([HD + 1, 512], FP32, name="st_b",
                                       tag="st")
                    nc.vector.tensor_copy(st_a[:, :], pv_a[:, :])
                    nc.vector.tensor_copy(st_b[:, :], pv_b[:, :])
                    queue.insert(0, make_epilogue2(qb_, p_, st_a, st_b))
                    del pv[(qb_, p_)]

            chunks = [(qb, p, kc) for qb in range(NQ) for p in range(2)
                      for kc in range(4 * (qb + 1))]
            pending = []
            last_qb = -1
            for qb, p, kc in chunks:
                if qb != last_qb:
                    for fn in prologue.get(qb, ()):
                        fn()
                    last_qb = qb
                pending.append((qb, p, kc) + emit_S_exp_mask(qb, p, kc))
                if len(pending) >= 3:
                    if queue:
                        queue.pop(0)()   # fill exp slack with queued work
                    flush_one()
            while pending:
                if queue:
                    queue.pop(0)()
                flush_one()

        def emit_body():
            queue = []
            # DMA priority: the first QK step needs wq8 + x8(0) only
            nc.sync.dma_start(out=wq8[:, :, :, :], in_=wq_d)
            emit_x8_dma(0, half=0)
            emit_x8_dma(0, half=1)
            nc.sync.dma_start(out=wk8[:, :, :, :], in_=wk_d)
            emit_xt_dma(0)
            emit_weights_vo()
            emit_x8_dma(1)
            emit_xt_dma(1)
            for s in make_qk_steps(0):
                s()
            for s in make_v_steps(0, tccs=[0]):
                s()
            # queue distribution balances each attention block's PE load
            # (S+PV+pops) against its exp budget (8*(qb+1) chunks): v(ib)
            # and outproj(qb) defer as late as deps allow -- PV touches
            # vt[kc] only ~kc chunks in, outproj(qb) only needs ctxT(qb)
            queue += make_v_steps(0, tccs=[1, 2, 3]) + make_qk_steps(1)
            emit_x8_dma(2)
            emit_xt_dma(2)

            def pro1():
                emit_x8_dma(3)
                emit_xt_dma(3)
                queue.extend(make_v_steps(1) + make_qk_steps(2))

            def pro2():
                queue.extend(make_v_steps(2) + make_qk_steps(3)
                             + make_outproj_steps(0))

            def pro3():
                queue.extend(make_v_steps(3) + make_outproj_steps(1)
                             + make_outproj_steps(2))

            emit_attention_stream(queue, {1: [pro1], 2: [pro2], 3: [pro3]})
            queue += make_outproj_steps(3)
            while queue:
                queue.pop(0)()

        for _rep in range(repeat):
            emit_body()

    nc.compile()
    return nc


def _get_nc(repeat=1):
    key = ("nc", repeat)
    if key not in _CACHE:
        _CACHE[key] = _build_nc(repeat)
    return _CACHE[key]


def _np_f8():
    import concourse.mybir as mybir
    return mybir.dt.np(mybir.dt.float8e4)


def _make_in_maps(x, Wq, Wk, Wv, Wo):
    import ml_dtypes
    bf = ml_dtypes.bfloat16
    f8 = _np_f8()
    x = np.asarray(x, dtype=np.float32)
    in_maps = []

    def dr_w(Wg):
        """[1024, 256] -> fp8 DoubleRow layout [128, 4, 2, 256] with
        columns permuted to (d_half, head, d_local%32)."""
        Wp = Wg.reshape(D, HG, 2, 32).transpose(0, 2, 1, 3).reshape(D, DG)
        return np.ascontiguousarray(
            Wp.reshape(NF // 2, 2, 128, DG).transpose(2, 0, 1, 3)).astype(f8)

    for c in range(NCORES):
        b, g = divmod(c, 4)
        cs = slice(DG * g, DG * (g + 1))
        xT = np.ascontiguousarray(x[b].T)                       # [1024, N]
        xt_bf = np.ascontiguousarray(
            xT.reshape(NF, 128, N).transpose(1, 0, 2)).astype(bf)
        x8 = np.ascontiguousarray(
            xT.reshape(NF // 2, 2, 128, N).transpose(2, 0, 1, 3)).astype(f8)
        wv = np.ascontiguousarray(
            np.asarray(Wv[:, cs], np.float32).reshape(NF, 128, DG)
            .transpose(1, 0, 2)).astype(bf)
        wo = np.ascontiguousarray(
            np.asarray(Wo[cs, :], np.float32).reshape(2, 128, D)
            .transpose(1, 0, 2)).astype(bf)
        in_maps.append({
            "x8": x8,
            "xt": xt_bf,
            "wq8": dr_w(32.0 * np.asarray(Wq[:, cs], np.float32)),
            "wk8": dr_w(32.0 * np.asarray(Wk[:, cs], np.float32)),
            "wv": wv,
            "wo": wo,
        })
    return in_maps


def _gather(results, bo):
    out = np.empty((2, N, D), dtype=np.float32)
    for b in range(2):
        acc = results[4 * b]["out"].astype(np.float32)
        for g in range(1, 4):
            acc = acc + results[4 * b + g]["out"].astype(np.float32)
        out[b] = acc + bo[None, :].astype(np.float32)
    return out


def run_spmd(x, Wq, Wk, Wv, Wo, bo, **spmd_kwargs):
    """Run the 8-core kernel; returns (full_output, BassKernelResults)."""
    from concourse.bass_utils import run_bass_kernel_spmd

    nc = _get_nc()
    in_maps = _make_in_maps(
        np.asarray(x), np.asarray(Wq), np.asarray(Wk), np.asarray(Wv),
        np.asarray(Wo))
    res = run_bass_kernel_spmd(nc, in_maps, core_ids=list(range(NCORES)),
                               **spmd_kwargs)
    return _gather(res.results, np.asarray(bo)), res


def kernel(x, Wq, Wk, Wv, Wo, bo):
    out, _ = run_spmd(x, Wq, Wk, Wv, Wo, bo)
    return out
